# revision 1
# baseline (speedup 1.0000x reference)
"""MaxIoUAssigner on 8 Trainium2 NeuronCores (Bass/Tile).

kernel(bboxes[200000,4] f32, gt_bboxes[256,4] f32) -> assigned[200000] int32

Reference semantics reproduced exactly:
  overlaps = iou(gt, priors)  [G=256, N=200000]
  per-prior max/argmax (first index wins ties); < 0.5 -> 0; >= 0.5 -> argmax+1
  low-quality: priors tying a gt's row max get gt_i+1 (later gt wins)

Distribution: priors sharded across 8 cores (25000 each, padded to 25600 =
10 chunks of 2560 with far-away zero-IoU dummy boxes). The per-gt row max
needs a cross-shard reduction: done on-device with a 1 KB DRAM AllReduce(max).

Layout (chosen for this platform's per-instruction-dominated cost model):
  - 256 gts -> 2 partition blocks of 128; gt coords/areas are per-partition
    scalars, so the whole IoU pipeline is fused tensor_scalar /
    scalar_tensor_tensor ops over [128, 2560] tiles.
  - prior coords+areas (areas precomputed on host, bit-identical f32) are
    0-stride-broadcast DMA'd into [128, 5, 2560] tiles: one DMA per chunk.
  - per-gt max = free-dim reduce; per-prior max / argmax / low-quality
    labels = partition_all_reduce (one GPSIMD instr per chunk each).
  - argmax-first tie-break: max over (256-g)*[iou==pmax]; low-quality
    later-gt-wins: max over (g+1)*[iou==gtmax].
  - IoU tiles stashed to DRAM between the two phases; exact (bit-accurate)
    nc.vector.reciprocal for the division.
"""

import sys

if "/opt/trn_rl_repo" not in sys.path:
    sys.path.insert(0, "/opt/trn_rl_repo")

import numpy as np

from concourse import bacc, bass_utils, mybir, tile

f32 = mybir.dt.float32
i32 = mybir.dt.int32
Alu = mybir.AluOpType

N_FULL = 200000
G = 256
GB = 2                               # gt partition blocks
P = 128
N_CORES = 8
N_SHARD = N_FULL // N_CORES          # 25000
F = 3200                             # priors per chunk
NS = 25600                           # padded shard (8 chunks)
PAD_BOX = (4000.0, 4000.0, 4001.0, 4001.0)


def build_program(ns=NS, n_cores=N_CORES, repeat=1, f=F):
    import concourse.bass_isa as bass_isa

    chunks = ns // f
    fs = f // P
    TS_ = chunks * fs
    nc = bacc.Bacc("TRN2", target_bir_lowering=False, debug=False,
                   num_devices=n_cores)
    bb = nc.dram_tensor("bb", [5, ns], f32, kind="ExternalInput").ap()
    gt = nc.dram_tensor("gt", [G, 4], f32, kind="ExternalInput").ap()
    out = nc.dram_tensor("assigned", [ns], i32, kind="ExternalOutput").ap()

    with tile.TileContext(nc) as tc:
        with (
            tc.tile_pool(name="const", bufs=1) as cpool,
            tc.tile_pool(name="work", bufs=1) as wpool,
            tc.tile_pool(name="rows", bufs=2) as rpool,
            tc.tile_pool(name="dram", bufs=1, space="DRAM") as dpool,
        ):
            # ---- constants ----
            gtc = cpool.tile([P, GB, 4], f32, tag="gtc")
            agc = cpool.tile([P, GB], f32, tag="agc")
            gw = cpool.tile([P, GB], f32, tag="gw")
            gh = cpool.tile([P, GB], f32, tag="gh")
            wrev_i = cpool.tile([P, GB], i32, tag="wrevi")
            wrev = cpool.tile([P, GB], f32, tag="wrev")
            gp1_i = cpool.tile([P, GB], i32, tag="gp1i")
            gp1 = cpool.tile([P, GB], f32, tag="gp1")
            gacc = cpool.tile([P, GB], f32, tag="gacc")
            gtmaxc = cpool.tile([P, GB], f32, tag="gtmaxc")
            pm_st = cpool.tile([P, TS_], f32, tag="pmst")
            am_st = cpool.tile([P, TS_], f32, tag="amst")
            lq_st = cpool.tile([P, TS_], f32, tag="lqst")
            cmb_m = cpool.tile([P, TS_], f32, tag="cmbm")
            cmb_v = cpool.tile([P, TS_], f32, tag="cmbv")
            out_i = cpool.tile([P, TS_], i32, tag="outi")

            stash = dpool.tile([G, ns], f32, tag="stash")
            st_dram = dpool.tile([3, ns], f32, tag="stdram")
            cc_in = dpool.tile([1, G], f32, tag="ccin")
            cc_out = dpool.tile([1, G], f32, tag="ccout")

            def bc1(col2, n):
                # [P, GB, n] 0-step-broadcast view of a [P, GB] column pair
                return (col2.rearrange("p (b o) -> p b o", o=1)
                        .broadcast_to([P, GB, n]))

            # gt g = b*128+p -> per-partition scalars
            nc.sync.dma_start(gtc[:], gt.rearrange("(b p) c -> p b c", p=P))
            nc.vector.tensor_sub(gw[:], gtc[:, :, 2], gtc[:, :, 0])
            nc.vector.tensor_sub(gh[:], gtc[:, :, 3], gtc[:, :, 1])
            nc.vector.tensor_mul(agc[:], gw[:], gh[:])
            # wrev[p,b] = 256-(b*128+p); gp1[p,b] = b*128+p+1
            nc.gpsimd.iota(wrev_i[:], pattern=[[-P, GB]], base=G,
                           channel_multiplier=-1)
            nc.vector.tensor_copy(wrev[:], wrev_i[:])
            nc.gpsimd.iota(gp1_i[:], pattern=[[P, GB]], base=1,
                           channel_multiplier=1)
            nc.vector.tensor_copy(gp1[:], gp1_i[:])
            nc.gpsimd.memset(gacc[:], 0.0)

            for _rep in range(repeat):
                # ---- phase 1: iou, per-gt max, per-prior max/argmax ----
                for c in range(chunks):
                    col = slice(c * f, (c + 1) * f)
                    b5 = wpool.tile([P, 5, f], f32, tag="b5")
                    nc.sync.dma_start(
                        b5[:], bb[:, col].rearrange("(o c) n -> o c n", o=1)
                        .broadcast_to([P, 5, f]))
                    bx1_t, by1_t = b5[:, 0], b5[:, 1]
                    bx2_t, by2_t = b5[:, 2], b5[:, 3]
                    ab_t = b5[:, 4]

                    ix_t = wpool.tile([P, f], f32, tag="ix")
                    iy_t = wpool.tile([P, f], f32, tag="iy")
                    s1_t = wpool.tile([P, f], f32, tag="s1")
                    s2_t = wpool.tile([P, f], f32, tag="s2")
                    t_a = wpool.tile([P, GB, f], f32, tag="ta")
                    u_a = wpool.tile([P, GB, f], f32, tag="ua")
                    r_a = wpool.tile([P, GB, f], f32, tag="b5")
                    iou_a = wpool.tile([P, GB, f], f32, tag="ioua")

                    for b in range(GB):
                        gx1 = gtc[:, b, 0:1]
                        gy1 = gtc[:, b, 1:2]
                        gx2 = gtc[:, b, 2:3]
                        gy2 = gtc[:, b, 3:4]
                        # lt = max(gt[:2], prior[:2])
                        nc.vector.tensor_scalar(ix_t[:], bx1_t, gx1, None,
                                                op0=Alu.max)
                        nc.vector.tensor_scalar(iy_t[:], by1_t, gy1, None,
                                                op0=Alu.max)
                        # s = min(gt[2:], prior[2:]) - lt
                        nc.vector.scalar_tensor_tensor(
                            s1_t[:], bx2_t, gx2, ix_t[:],
                            op0=Alu.min, op1=Alu.subtract)
                        nc.vector.scalar_tensor_tensor(
                            s2_t[:], by2_t, gy2, iy_t[:],
                            op0=Alu.min, op1=Alu.subtract)
                        # t = max(s1,0)*s2 (<=0 where no overlap; every
                        # downstream comparison matches reference's 0)
                        nc.vector.scalar_tensor_tensor(
                            t_a[:, b], s1_t[:], 0.0, s2_t[:],
                            op0=Alu.max, op1=Alu.mult)
                        # u = (area_b + area_g) - t  (f32 add commutes bitwise)
                        nc.vector.scalar_tensor_tensor(
                            u_a[:, b], ab_t, agc[:, b:b + 1], t_a[:, b],
                            op0=Alu.add, op1=Alu.subtract)

                    nc.vector.reciprocal(r_a.rearrange("p b n -> p (b n)"),
                                         u_a.rearrange("p b n -> p (b n)"))
                    nc.vector.tensor_mul(iou_a[:], t_a[:], r_a[:])

                    # per-gt running max
                    gred = rpool.tile([P, GB], f32, tag="gred")
                    nc.vector.tensor_reduce(gred[:], iou_a[:],
                                            axis=mybir.AxisListType.X,
                                            op=Alu.max)
                    nc.vector.tensor_max(gacc[:], gacc[:], gred[:])

                    # stash iou (gt-major [256, ns]) for phase 2
                    nc.sync.dma_start(
                        stash[:, col].rearrange("(b p) n -> p b n", p=P),
                        iou_a[:])

                    # per-prior max over gts
                    pr_a = wpool.tile([P, GB, f], f32, tag="b5")
                    nc.gpsimd.partition_all_reduce(
                        pr_a.rearrange("p b n -> p (b n)"),
                        iou_a.rearrange("p b n -> p (b n)"),
                        channels=P, reduce_op=bass_isa.ReduceOp.max)
                    pam = wpool.tile([P, 2, f], f32, tag="ua")
                    pmax_t = pam[:, 0]
                    nc.vector.tensor_max(pmax_t, pr_a[:, 0], pr_a[:, 1])

                    # argmax-first: max of (256-g)*[iou==pmax]
                    msk_a = wpool.tile([P, GB, f], f32, tag="b5")
                    nc.vector.tensor_tensor(
                        msk_a[:], iou_a[:],
                        pmax_t.rearrange("p (o n) -> p o n", o=1)
                        .broadcast_to([P, GB, f]),
                        op=Alu.is_ge)
                    nc.vector.tensor_mul(msk_a[:], msk_a[:], bc1(wrev[:], f))
                    nc.gpsimd.partition_all_reduce(
                        msk_a.rearrange("p b n -> p (b n)"),
                        msk_a.rearrange("p b n -> p (b n)"),
                        channels=P, reduce_op=bass_isa.ReduceOp.max)
                    nc.vector.tensor_max(pam[:, 1], msk_a[:, 0], msk_a[:, 1])

                    # stage pmax+argmax rows in one DMA (row 0 = full result)
                    nc.sync.dma_start(
                        st_dram[0:2, col].rearrange("(o b) n -> o b n", o=1),
                        pam[0:1, :, :])

                # ---- all-reduce per-gt max across the 8 cores ----
                nc.sync.dma_start(
                    cc_in.rearrange("o (b p) -> (o p) b", p=P), gacc[:])
                nc.gpsimd.collective_compute(
                    "AllReduce", Alu.max,
                    replica_groups=[list(range(n_cores))],
                    ins=[cc_in[:].opt()], outs=[cc_out[:].opt()])
                nc.sync.dma_start(
                    gtmaxc[:], cc_out.rearrange("o (b p) -> (o p) b", p=P))

                # ---- phase 2: low-quality matches from stashed iou ----
                for c in range(chunks):
                    col = slice(c * f, (c + 1) * f)
                    iou_a = wpool.tile([P, GB, f], f32, tag="ioua")
                    cd_a = wpool.tile([P, GB, f], f32, tag="b5")
                    nc.sync.dma_start(
                        iou_a[:],
                        stash[:, col].rearrange("(b p) n -> p b n", p=P))
                    # iou == gtmax  <=>  iou >= gtmax (iou <= gtmax always)
                    nc.vector.tensor_tensor(cd_a[:], iou_a[:],
                                            bc1(gtmaxc[:], f), op=Alu.is_ge)
                    nc.vector.tensor_mul(cd_a[:], cd_a[:], bc1(gp1[:], f))
                    nc.gpsimd.partition_all_reduce(
                        cd_a.rearrange("p b n -> p (b n)"),
                        cd_a.rearrange("p b n -> p (b n)"),
                        channels=P, reduce_op=bass_isa.ReduceOp.max)
                    lq_t = wpool.tile([P, 2, f], f32, tag="ua")
                    nc.vector.tensor_max(lq_t[:, 0], cd_a[:, 0], cd_a[:, 1])
                    nc.sync.dma_start(st_dram[2:3, col], lq_t[0:1, 0, :])

            # reload staged rows as [128, chunks*fs]
            for v, tl in ((0, pm_st), (1, am_st), (2, lq_st)):
                nc.sync.dma_start(
                    tl[:].rearrange("p (c f) -> p c f", f=fs),
                    st_dram[v, :].rearrange("(c p f) -> p c f", p=P, f=fs))

            # ---- combine: lq > 0 ? lq : (pmax >= 0.5 ? (257-am) : 0) ----
            nc.vector.tensor_scalar(cmb_m[:], pm_st[:], 0.5, None,
                                    op0=Alu.is_ge)
            nc.vector.tensor_scalar(cmb_v[:], am_st[:], -1.0, float(G + 1),
                                    op0=Alu.mult, op1=Alu.add)
            nc.vector.tensor_mul(cmb_v[:], cmb_v[:], cmb_m[:])
            nc.vector.tensor_scalar(cmb_m[:], lq_st[:], 1.0, None,
                                    op0=Alu.is_lt)
            nc.vector.tensor_mul(cmb_v[:], cmb_v[:], cmb_m[:])
            nc.vector.tensor_add(cmb_v[:], cmb_v[:], lq_st[:])
            nc.vector.tensor_copy(out_i[:], cmb_v[:])
            nc.sync.dma_start(
                out.rearrange("(c p f) -> p c f", p=P, f=fs),
                out_i[:].rearrange("p (c f) -> p c f", f=fs))

    nc.compile()
    return nc


def make_bbx(shard_boxes, ns):
    """[n,4] f32 -> [5, ns]: rows x1,y1,x2,y2,area; PAD_BOX padding."""
    n = shard_boxes.shape[0]
    bbx = np.empty((5, ns), np.float32)
    bbx[0, :n] = shard_boxes[:, 0]
    bbx[1, :n] = shard_boxes[:, 1]
    bbx[2, :n] = shard_boxes[:, 2]
    bbx[3, :n] = shard_boxes[:, 3]
    pb = np.array(PAD_BOX, np.float32)
    bbx[0, n:], bbx[1, n:], bbx[2, n:], bbx[3, n:] = pb[0], pb[1], pb[2], pb[3]
    bbx[4] = (bbx[2] - bbx[0]) * (bbx[3] - bbx[1])
    return bbx


_NC_CACHE = None


def _get_program():
    global _NC_CACHE
    if _NC_CACHE is None:
        _NC_CACHE = build_program()
    return _NC_CACHE


def kernel(bboxes: np.ndarray, gt_bboxes: np.ndarray) -> np.ndarray:
    assert bboxes.shape == (N_FULL, 4) and gt_bboxes.shape == (G, 4)
    nc = _get_program()

    bboxes = np.ascontiguousarray(bboxes, dtype=np.float32)
    gt = np.ascontiguousarray(gt_bboxes, dtype=np.float32)
    in_maps = []
    for c in range(N_CORES):
        shard = bboxes[c * N_SHARD:(c + 1) * N_SHARD]
        in_maps.append({"bb": make_bbx(shard, NS), "gt": gt})

    res = bass_utils.run_bass_kernel_spmd(nc, in_maps,
                                          core_ids=list(range(N_CORES)))
    outs = [res.results[c]["assigned"][:N_SHARD] for c in range(N_CORES)]
    return np.concatenate(outs).astype(np.int32)


if __name__ == "__main__":
    rng = np.random.default_rng(0)
    bb_ = np.zeros((N_FULL, 4), np.float32)
    bb_[:, :2] = rng.uniform(0, 928, (N_FULL, 2))
    bb_[:, 2:] = bb_[:, :2] + rng.uniform(1, 97, (N_FULL, 2))
    gtb = np.zeros((G, 4), np.float32)
    gtb[:, :2] = rng.uniform(0, 928, (G, 2))
    gtb[:, 2:] = gtb[:, :2] + rng.uniform(1, 97, (G, 2))
    print(kernel(bb_, gtb)[:20])



# revision 5
# speedup vs baseline: 4.2199x; 4.2199x over previous
"""MaxIoUAssigner on 8 Trainium2 NeuronCores (Bass/Tile) — v2.

kernel(bboxes[200000,4] f32, gt_bboxes[256,4] f32) -> assigned[200000] int32

Semantics (mmdet MaxIoUAssigner, pos=neg=0.5, min_pos_iou=0,
gt_max_assign_all=True):
  overlaps = iou(gt, priors)  [256, 200000]
  per-prior max/argmax (first index wins ties); <0.5 -> 0; >=0.5 -> argmax+1
  low-quality: priors attaining a gt's row max get gt_i+1 (later gt wins)

This target's cost model is dominated by a flat ~41us per *instruction*
(engine-independent, size-independent); DMA bytes add ~(1/100GB/s).  The
kernel is therefore architected to minimize instruction count:

 - Priors are sorted by x on the host and split into 8 contiguous x-bands
   (one per core, 25000 each); each band is split by y into two halves.
   Each half only interacts with the <=64 gts whose boxes can touch it
   (data-checked; host falls back to a numpy path if a cap is exceeded).
 - On-device layout: 128 partitions = 2 groups x 64 gts; free dim = that
   half's priors (2 chunks x 6250).  Broadcast DMA amplification drops
   from 128x to 64x and the whole IoU core is 8 fat instructions/chunk.
 - Per-prior argmax+max: iou is encoded as key = floor(iou*2^15) +
   (63-p)/64 (exact in f32: 16+6 bits) and max-reduced across partitions
   with one gpsimd partition_all_reduce per group; the host decodes
   (bucketed argmax, exact 0.5 threshold via the floor).
 - Per-gt max+argmax (for the low-quality step) via the DVE max/max_index
   top-8 instruction pair per chunk; candidates are combined on the host
   across chunks/cores, which removes the iou stash, the whole second
   phase, and the gt-max AllReduce collective of the v1 design.
 - Total ~40 instructions per core vs ~250 in v1.

Host does only O(N) label decode + argsort; all 51.2M-element IoU work
runs on device.
"""

import sys

if "/opt/trn_rl_repo" not in sys.path:
    sys.path.insert(0, "/opt/trn_rl_repo")

import numpy as np

from concourse import bacc, bass_utils, mybir, tile

f32 = mybir.dt.float32
i32 = mybir.dt.int32
u32 = mybir.dt.uint32
Alu = mybir.AluOpType
ActF = mybir.ActivationFunctionType

N_FULL = 200000
G = 256
P = 128
HG = 64                      # gts per group (2 groups of 64 partitions)
N_CORES = 8
NB = N_FULL // N_CORES       # 25000 priors per core (x-band)
NH = NB // 2                 # 12500 per y-half
CH = 2                       # chunks per half
F = NH // CH                 # 6250 priors per chunk
KSCALE = 32768.0             # 2^15 iou quantization for the key encode
KTHR = 16384                 # floor(iou*2^15) >= 16384  <=>  iou >= 0.5
DUMMY = 1.0e8                # far-away dummy gt coordinate


def build_program(repeat=1, n_cores=N_CORES):
    import concourse.bass_isa as bass_isa

    nc = bacc.Bacc("TRN2", target_bir_lowering=False, debug=False,
                   num_devices=n_cores)
    bbx = nc.dram_tensor("bbx", [2, 5, NH], f32, kind="ExternalInput").ap()
    gtc_d = nc.dram_tensor("gtc", [P, 8], f32, kind="ExternalInput").ap()
    okey = nc.dram_tensor("okey", [CH, 2, F], f32, kind="ExternalOutput").ap()
    ogvi = nc.dram_tensor("ogvi", [P, 16 * CH], u32,
                          kind="ExternalOutput").ap()

    with tile.TileContext(nc) as tc:
        with (
            tc.tile_pool(name="c", bufs=1) as cpool,
            tc.tile_pool(name="w", bufs=1) as wpool,
        ):
            gtc = cpool.tile([P, 8], f32, tag="gtc")
            gvi = cpool.tile([P, 16 * CH], u32, tag="gvi")
            io = wpool.tile([P, F], f32, tag="io")         # 25 KB
            tmp = wpool.tile([P, F], f32, tag="tmp")       # 25 KB
            tmp2 = wpool.tile([HG, F], f32, tag="tmp2")    # 12.5 KB

            nc.sync.dma_start(gtc[:], gtc_d)
            gx1, gx2 = gtc[:, 0:1], gtc[:, 1:2]
            gy1, gy2 = gtc[:, 2:3], gtc[:, 3:4]
            gar, frac = gtc[:, 4:5], gtc[:, 5:6]

            for _rep in range(repeat):
                for c in range(CH):
                    col = slice(c * F, (c + 1) * F)
                    cb = wpool.tile([P, 5, F], f32, tag="cb")  # 125 KB
                    for h in range(2):
                        nc.sync.dma_start(
                            cb[h * HG:(h + 1) * HG],
                            bbx[h, :, col].rearrange("r n -> () r n")
                            .broadcast_to([HG, 5, F]))
                    # iou core: 8 instructions
                    nc.vector.tensor_scalar(tmp[:], cb[:, 0], gx1, None,
                                            op0=Alu.max)
                    nc.vector.scalar_tensor_tensor(io[:], cb[:, 1], gx2,
                                                   tmp[:], op0=Alu.min,
                                                   op1=Alu.subtract)
                    nc.vector.tensor_scalar(tmp[:], cb[:, 2], gy1, None,
                                            op0=Alu.max)
                    nc.vector.scalar_tensor_tensor(tmp[:], cb[:, 3], gy2,
                                                   tmp[:], op0=Alu.min,
                                                   op1=Alu.subtract)
                    nc.vector.scalar_tensor_tensor(io[:], io[:], 0.0, tmp[:],
                                                   op0=Alu.max, op1=Alu.mult)
                    nc.vector.scalar_tensor_tensor(tmp[:], cb[:, 4], gar,
                                                   io[:], op0=Alu.add,
                                                   op1=Alu.subtract)
                    nc.vector.reciprocal(tmp[:], tmp[:])
                    nc.vector.tensor_mul(io[:], io[:], tmp[:])
                    # per-gt top8 values + indices for the low-quality step
                    vof, iof = 16 * c, 16 * c + 8
                    nc.vector.max(gvi.bitcast(f32)[:, vof:vof + 8], io[:])
                    nc.vector.max_index(gvi[:, iof:iof + 8],
                                        gvi.bitcast(f32)[:, vof:vof + 8],
                                        io[:])
                    # per-prior key encode: floor(iou*2^15) + (63-p)/64
                    ki = wpool.tile([P, F], i32, tag="cb")  # reuse cb slot
                    nc.scalar.activation(ki[:], io[:], ActF.Copy, bias=-0.5,
                                         scale=KSCALE)
                    nc.scalar.activation(tmp[:], ki[:], ActF.Relu, bias=frac,
                                         scale=1.0)
                    # group max across partitions (AR ucode is base-0
                    # only: copy group B down to partition 0 first)
                    nc.sync.dma_start(tmp2[:], tmp[HG:P])
                    nc.gpsimd.partition_all_reduce(
                        tmp[0:HG], tmp[0:HG], channels=HG,
                        reduce_op=bass_isa.ReduceOp.max)
                    nc.gpsimd.partition_all_reduce(
                        tmp2[:], tmp2[:], channels=HG,
                        reduce_op=bass_isa.ReduceOp.max)
                    nc.sync.dma_start(okey[c, 0:1, :], tmp[0:1, :])
                    nc.sync.dma_start(okey[c, 1:2, :], tmp2[0:1, :])
            nc.sync.dma_start(ogvi, gvi[:])
    nc.compile()
    return nc


_NC_CACHE = None


def _get_program():
    global _NC_CACHE
    if _NC_CACHE is None:
        _NC_CACHE = build_program()
    return _NC_CACHE


def prepare_inputs(bb, gt):
    """Sort priors into 8 x-bands x 2 y-halves; pick each half's gts.

    Returns (in_maps, meta) where meta[k] = (halves_idx, gmaps):
    halves_idx[h] = global prior indices of half h (device column order),
    gmaps[h] = ascending global gt indices assigned to that half's group.
    Returns None if a gt group exceeds HG (caller falls back).
    """
    xorder = np.argsort(bb[:, 0], kind="stable")
    in_maps, meta = [], []
    for k in range(N_CORES):
        band_idx = xorder[k * NB:(k + 1) * NB]
        yord = np.argsort(bb[band_idx, 1], kind="stable")
        halves = [band_idx[yord[:NH]], band_idx[yord[NH:]]]
        bbx = np.empty((2, 5, NH), np.float32)
        gtc = np.zeros((P, 8), np.float32)
        gmaps = []
        for h in range(2):
            B = bb[halves[h]]
            bbx[h, 0] = B[:, 0]
            bbx[h, 1] = B[:, 2]
            bbx[h, 2] = B[:, 1]
            bbx[h, 3] = B[:, 3]
            bbx[h, 4] = (B[:, 2] - B[:, 0]) * (B[:, 3] - B[:, 1])
            sel = np.nonzero(
                (gt[:, 0] <= B[:, 2].max()) & (gt[:, 2] >= B[:, 0].min())
                & (gt[:, 1] <= B[:, 3].max()) & (gt[:, 3] >= B[:, 1].min())
            )[0]
            if len(sel) > HG:
                return None
            base = h * HG
            n = len(sel)
            gtc[base:base + n, 0] = gt[sel, 0]
            gtc[base:base + n, 1] = gt[sel, 2]
            gtc[base:base + n, 2] = gt[sel, 1]
            gtc[base:base + n, 3] = gt[sel, 3]
            gtc[base:base + n, 4] = ((gt[sel, 2] - gt[sel, 0])
                                     * (gt[sel, 3] - gt[sel, 1]))
            gtc[base + n:base + HG, 0] = DUMMY
            gtc[base + n:base + HG, 1] = DUMMY + 1.0
            gtc[base + n:base + HG, 2] = DUMMY
            gtc[base + n:base + HG, 3] = DUMMY + 1.0
            gtc[base + n:base + HG, 4] = 1.0
            gtc[base:base + HG, 5] = (HG - 1 - np.arange(HG)) / HG
            gmaps.append(sel)
        in_maps.append({"bbx": bbx, "gtc": gtc})
        meta.append((halves, gmaps))
    return in_maps, meta


def assemble(res, meta):
    """Decode per-prior keys + per-gt candidates into final labels."""
    assigned = np.zeros(N_FULL, np.int32)
    cand = [[] for _ in range(G)]  # per gt: list of (val, prior) candidates
    for k in range(N_CORES):
        halves, gmaps = meta[k]
        r = res.results[k]
        okey = r["okey"]                       # [CH, 2, F]
        ogvi = r["ogvi"]                       # [P, 16*CH] u32
        for c in range(CH):
            for h in range(2):
                v = okey[c, h].astype(np.float64)
                w = np.rint(v * HG).astype(np.int64)
                kib = w >> 6
                plocal = (HG - 1) - (w & (HG - 1))
                gsel = gmaps[h]
                gl = np.full(HG, -1, np.int64)
                gl[:len(gsel)] = gsel
                gwin = gl[np.clip(plocal, 0, HG - 1)]
                lab = np.where((kib >= KTHR) & (gwin >= 0), gwin + 1, 0)
                assigned[halves[h][c * F:(c + 1) * F]] = lab
        # gt-side candidates
        for c in range(CH):
            val = ogvi[:, 16 * c:16 * c + 8].view(np.float32)
            idx = ogvi[:, 16 * c + 8:16 * c + 16]
            for h in range(2):
                gsel = gmaps[h]
                if not len(gsel):
                    continue
                base = h * HG
                pri = halves[h][c * F:(c + 1) * F]
                for pl, g in enumerate(gsel):
                    v0 = val[base + pl, 0]
                    cand[g].append((v0, pri[idx[base + pl, 0]]))
                    # exact ties within the chunk's top-8
                    j = 1
                    while j < 8 and val[base + pl, j] == v0:
                        cand[g].append((v0, pri[idx[base + pl, j]]))
                        j += 1
    for g in range(G):
        if not cand[g]:
            continue
        vmax = max(v for v, _ in cand[g])
        for v, p in cand[g]:
            if v == vmax:
                assigned[p] = g + 1
    return assigned


def _host_fallback(bb, gt):
    """Pure-numpy reference path (used only if a gt-group cap is hit)."""
    N = bb.shape[0]
    max_ov = np.zeros(N, np.float32)
    arg_ov = np.zeros(N, np.int64)
    gt_max = np.zeros(G, np.float32)
    gt_arg = np.zeros(G, np.int64)
    area_g = (gt[:, 2] - gt[:, 0]) * (gt[:, 3] - gt[:, 1])
    area_b = (bb[:, 2] - bb[:, 0]) * (bb[:, 3] - bb[:, 1])
    step = 20000
    best = np.full(N, -np.inf, np.float32)
    for s in range(0, N, step):
        e = min(s + step, N)
        lt = np.maximum(gt[:, None, :2], bb[None, s:e, :2])
        rb = np.minimum(gt[:, None, 2:], bb[None, s:e, 2:])
        wh = np.clip(rb - lt, 0, None).astype(np.float32)
        inter = wh[..., 0] * wh[..., 1]
        union = np.maximum(area_g[:, None] + area_b[None, s:e] - inter,
                           np.float32(1e-6))
        ov = inter / union
        max_ov[s:e] = ov.max(axis=0)
        arg_ov[s:e] = ov.argmax(axis=0)
        cm = ov.max(axis=1)
        upd = cm > gt_max
        gt_max = np.where(upd, cm, gt_max)
        gt_arg = np.where(upd, s + ov.argmax(axis=1), gt_arg)
        del lt, rb, wh, inter, union, ov
    assigned = np.where(max_ov >= 0.5, arg_ov + 1, 0).astype(np.int32)
    # low-quality (ties beyond the argmax are vanishingly rare; replicate
    # reference exactly by re-scanning rows for equality)
    for s in range(0, N, step):
        e = min(s + step, N)
        lt = np.maximum(gt[:, None, :2], bb[None, s:e, :2])
        rb = np.minimum(gt[:, None, 2:], bb[None, s:e, 2:])
        wh = np.clip(rb - lt, 0, None).astype(np.float32)
        inter = wh[..., 0] * wh[..., 1]
        union = np.maximum(area_g[:, None] + area_b[None, s:e] - inter,
                           np.float32(1e-6))
        ov = inter / union
        eq = ov == gt_max[:, None]
        gidx = np.where(eq, np.arange(G)[:, None], -1).max(axis=0)
        sel = gidx >= 0
        assigned[s:e][sel] = gidx[sel] + 1
        del lt, rb, wh, inter, union, ov
    return assigned


def kernel(bboxes: np.ndarray, gt_bboxes: np.ndarray) -> np.ndarray:
    assert bboxes.shape == (N_FULL, 4) and gt_bboxes.shape == (G, 4)
    bb = np.ascontiguousarray(bboxes, dtype=np.float32)
    gt = np.ascontiguousarray(gt_bboxes, dtype=np.float32)
    prep = prepare_inputs(bb, gt)
    if prep is None:
        return _host_fallback(bb, gt)
    in_maps, meta = prep
    nc = _get_program()
    res = bass_utils.run_bass_kernel_spmd(nc, in_maps,
                                          core_ids=list(range(N_CORES)))
    return assemble(res, meta)


if __name__ == "__main__":
    rng = np.random.default_rng(0)
    bb_ = np.zeros((N_FULL, 4), np.float32)
    bb_[:, :2] = rng.uniform(0, 928, (N_FULL, 2))
    bb_[:, 2:] = bb_[:, :2] + rng.uniform(1, 97, (N_FULL, 2))
    gtb = np.zeros((G, 4), np.float32)
    gtb[:, :2] = rng.uniform(0, 928, (G, 2))
    gtb[:, 2:] = gtb[:, :2] + rng.uniform(1, 97, (G, 2))
    print(kernel(bb_, gtb)[:20])


# revision 9
# speedup vs baseline: 7.0638x; 1.6739x over previous
"""MaxIoUAssigner on 8 Trainium2 NeuronCores (Bass/Tile) — v3.

kernel(bboxes[200000,4] f32, gt_bboxes[256,4] f32) -> assigned[200000] int32

Semantics (mmdet MaxIoUAssigner, pos=neg=0.5, min_pos_iou=0,
gt_max_assign_all=True):
  overlaps = iou(gt, priors)  [256, 200000]
  per-prior max/argmax (first index wins ties); <0.5 -> 0; >=0.5 -> argmax+1
  low-quality: priors attaining a gt's row max get gt_i+1 (later gt wins)

This target's cost model is dominated by a flat ~41us per *instruction*
(engine-independent, nearly size-independent); DMA bytes overlap compute.
The kernel is therefore architected to minimize instruction count:

 - Priors are sorted by x on the host and split into 8 contiguous x-bands
   (one per core, 25000 each); each band is split by y into two halves.
   Each half only interacts with the <=64 gts whose boxes can touch it
   (data-checked; host falls back to a numpy path if a cap is exceeded).
 - On-device layout: 128 partitions = 2 groups x 64 gts; free dim = the
   half's full 12500 priors in ONE chunk.  Coordinates roll through a
   single [128,2,12500] buffer (x-pair, then y-pair, then area), so the
   whole IoU core is 8 fat instructions on [128,12500] operands.
 - Per-prior argmax+max: iou is encoded as key = floor(iou*2^15) +
   (63-p)/64 (exact in f32: 16+6 bits) and max-reduced across partitions
   with gpsimd partition_all_reduce per group; the host decodes
   (bucketed argmax, exact 0.5 threshold via the floor bias).
 - Per-gt max+argmax (for the low-quality step) via the DVE max/max_index
   top-8 instruction pair; candidates are combined on the host across
   halves/cores, which removes the iou stash, the whole second phase,
   and the gt-max AllReduce collective of the v1 design.
 - ~25 instructions per core vs ~250 in v1.

Host does only O(N) label decode + argsort; all 51.2M-element IoU work
runs on device.
"""

import sys

if "/opt/trn_rl_repo" not in sys.path:
    sys.path.insert(0, "/opt/trn_rl_repo")

import numpy as np

from concourse import bacc, bass_utils, mybir, tile

f32 = mybir.dt.float32
i32 = mybir.dt.int32
u32 = mybir.dt.uint32
Alu = mybir.AluOpType
ActF = mybir.ActivationFunctionType

N_FULL = 200000
G = 256
P = 128
HG = 64                      # gts per group (2 groups of 64 partitions)
N_CORES = 8
NB = N_FULL // N_CORES       # 25000 priors per core (x-band)
NH = NB // 2                 # 12500 per y-half (one chunk)
F = NH
KSCALE = 32768.0             # 2^15 iou quantization for the key encode
KTHR = 16384                 # floor(iou*2^15) >= 16384  <=>  iou >= 0.5
DUMMY = 1.0e8                # far-away dummy gt coordinate


def build_program(repeat=1, n_cores=N_CORES):
    import concourse.bass_isa as bass_isa

    nc = bacc.Bacc("TRN2", target_bir_lowering=False, debug=False,
                   num_devices=n_cores)
    # rows per half: x1, x2, y1, y2, area
    bbx = nc.dram_tensor("bbx", [2, 5, NH], f32, kind="ExternalInput").ap()
    gtc_d = nc.dram_tensor("gtc", [P, 8], f32, kind="ExternalInput").ap()
    okey = nc.dram_tensor("okey", [2, NH], f32, kind="ExternalOutput").ap()
    ogvi = nc.dram_tensor("ogvi", [P, 16], u32, kind="ExternalOutput").ap()

    with tile.TileContext(nc) as tc:
        with (
            tc.tile_pool(name="c", bufs=1) as cpool,
            tc.tile_pool(name="w", bufs=1) as wpool,
        ):
            gtc = cpool.tile([P, 8], f32, tag="gtc")
            gvi = cpool.tile([P, 16], u32, tag="gvi")

            nc.sync.dma_start(gtc[:], gtc_d)
            gx1, gx2 = gtc[:, 0:1], gtc[:, 1:2]
            gy1, gy2 = gtc[:, 2:3], gtc[:, 3:4]
            gar, frac = gtc[:, 4:5], gtc[:, 5:6]

            for _rep in range(repeat):
                pair = wpool.tile([P, 2, F], f32, tag="pair")  # 100 KB
                io = wpool.tile([P, F], f32, tag="io")         # 50 KB
                tmp = wpool.tile([P, F], f32, tag="tmp")       # 50 KB

                def ld(r0, r1, dst, dw):
                    # broadcast rows [r0:r1) of each half to its 64 parts
                    for h in range(2):
                        nc.sync.dma_start(
                            dst[h * HG:(h + 1) * HG, 0:dw],
                            bbx[h, r0:r1].rearrange("r n -> () r n")
                            .broadcast_to([HG, dw, F]))

                ld(0, 2, pair, 2)                              # x1, x2
                nc.vector.tensor_scalar(tmp[:], pair[:, 0], gx1, None,
                                        op0=Alu.max)
                nc.vector.scalar_tensor_tensor(io[:], pair[:, 1], gx2,
                                               tmp[:], op0=Alu.min,
                                               op1=Alu.subtract)
                ld(2, 4, pair, 2)                              # y1, y2
                nc.vector.tensor_scalar(tmp[:], pair[:, 0], gy1, None,
                                        op0=Alu.max)
                nc.vector.scalar_tensor_tensor(tmp[:], pair[:, 1], gy2,
                                               tmp[:], op0=Alu.min,
                                               op1=Alu.subtract)
                nc.vector.scalar_tensor_tensor(io[:], io[:], 0.0, tmp[:],
                                               op0=Alu.max, op1=Alu.mult)
                ld(4, 5, pair, 1)                              # area
                nc.vector.scalar_tensor_tensor(tmp[:], pair[:, 0], gar,
                                               io[:], op0=Alu.add,
                                               op1=Alu.subtract)
                nc.vector.reciprocal(tmp[:], tmp[:])
                nc.vector.tensor_mul(io[:], io[:], tmp[:])
                # per-gt top8 values + indices for the low-quality step
                nc.vector.max(gvi.bitcast(f32)[:, 0:8], io[:])
                nc.vector.max_index(gvi[:, 8:16],
                                    gvi.bitcast(f32)[:, 0:8], io[:])
                # per-prior key encode: floor(iou*2^15) + (63-p)/64
                ki = wpool.tile([P, F], i32, tag="pair")  # reuse pair slot
                nc.scalar.activation(ki[:], io[:], ActF.Copy, bias=-0.5,
                                     scale=KSCALE)
                nc.scalar.activation(tmp[:], ki[:], ActF.Relu, bias=frac,
                                     scale=1.0)
                # group max across partitions (AR ucode is base-0 only:
                # copy group B down to partition 0; io's slot is free now)
                tmp2 = wpool.tile([HG, F], f32, tag="io")
                nc.sync.dma_start(tmp2[:], tmp[HG:P])
                nc.gpsimd.partition_all_reduce(
                    tmp[0:HG], tmp[0:HG], channels=HG,
                    reduce_op=bass_isa.ReduceOp.max)
                nc.gpsimd.partition_all_reduce(
                    tmp2[:], tmp2[:], channels=HG,
                    reduce_op=bass_isa.ReduceOp.max)
                nc.sync.dma_start(okey[0:1, :], tmp[0:1, :])
                nc.sync.dma_start(okey[1:2, :], tmp2[0:1, :])
            nc.sync.dma_start(ogvi, gvi[:])
    nc.compile()
    return nc


_NC_CACHE = None


def _get_program():
    global _NC_CACHE
    if _NC_CACHE is None:
        _NC_CACHE = build_program()
    return _NC_CACHE


def prepare_inputs(bb, gt):
    """Sort priors into 8 x-bands x 2 y-halves; pick each half's gts.

    Returns (in_maps, meta) where meta[k] = (halves_idx, gmaps):
    halves_idx[h] = global prior indices of half h (device column order),
    gmaps[h] = ascending global gt indices assigned to that half's group.
    Returns None if a gt group exceeds HG (caller falls back).
    """
    xorder = np.argsort(bb[:, 0], kind="stable")
    in_maps, meta = [], []
    for k in range(N_CORES):
        band_idx = xorder[k * NB:(k + 1) * NB]
        yord = np.argsort(bb[band_idx, 1], kind="stable")
        halves = [band_idx[yord[:NH]], band_idx[yord[NH:]]]
        bbx = np.empty((2, 5, NH), np.float32)
        gtc = np.zeros((P, 8), np.float32)
        gmaps = []
        for h in range(2):
            B = bb[halves[h]]
            bbx[h, 0] = B[:, 0]
            bbx[h, 1] = B[:, 2]
            bbx[h, 2] = B[:, 1]
            bbx[h, 3] = B[:, 3]
            bbx[h, 4] = (B[:, 2] - B[:, 0]) * (B[:, 3] - B[:, 1])
            sel = np.nonzero(
                (gt[:, 0] <= B[:, 2].max()) & (gt[:, 2] >= B[:, 0].min())
                & (gt[:, 1] <= B[:, 3].max()) & (gt[:, 3] >= B[:, 1].min())
            )[0]
            if len(sel) > HG:
                return None
            base = h * HG
            n = len(sel)
            gtc[base:base + n, 0] = gt[sel, 0]
            gtc[base:base + n, 1] = gt[sel, 2]
            gtc[base:base + n, 2] = gt[sel, 1]
            gtc[base:base + n, 3] = gt[sel, 3]
            gtc[base:base + n, 4] = ((gt[sel, 2] - gt[sel, 0])
                                     * (gt[sel, 3] - gt[sel, 1]))
            gtc[base + n:base + HG, 0] = DUMMY
            gtc[base + n:base + HG, 1] = DUMMY + 1.0
            gtc[base + n:base + HG, 2] = DUMMY
            gtc[base + n:base + HG, 3] = DUMMY + 1.0
            gtc[base + n:base + HG, 4] = 1.0
            gtc[base:base + HG, 5] = (HG - 1 - np.arange(HG)) / HG
            gmaps.append(sel)
        in_maps.append({"bbx": bbx, "gtc": gtc})
        meta.append((halves, gmaps))
    return in_maps, meta


def assemble(res, meta):
    """Decode per-prior keys + per-gt candidates into final labels."""
    assigned = np.zeros(N_FULL, np.int32)
    cand = [[] for _ in range(G)]  # per gt: list of (val, prior) candidates
    for k in range(N_CORES):
        halves, gmaps = meta[k]
        r = res.results[k]
        okey = r["okey"]                       # [2, NH]
        ogvi = r["ogvi"]                       # [P, 16] u32
        for h in range(2):
            v = okey[h].astype(np.float64)
            w = np.rint(v * HG).astype(np.int64)
            kib = w >> 6
            plocal = (HG - 1) - (w & (HG - 1))
            gsel = gmaps[h]
            gl = np.full(HG, -1, np.int64)
            gl[:len(gsel)] = gsel
            gwin = gl[np.clip(plocal, 0, HG - 1)]
            lab = np.where((kib >= KTHR) & (gwin >= 0), gwin + 1, 0)
            assigned[halves[h]] = lab
        # gt-side candidates
        val = ogvi[:, 0:8].view(np.float32)
        idx = ogvi[:, 8:16]
        for h in range(2):
            gsel = gmaps[h]
            if not len(gsel):
                continue
            base = h * HG
            pri = halves[h]
            for pl, g in enumerate(gsel):
                v0 = val[base + pl, 0]
                cand[g].append((v0, pri[idx[base + pl, 0]]))
                # exact ties within this half's top-8
                j = 1
                while j < 8 and val[base + pl, j] == v0:
                    cand[g].append((v0, pri[idx[base + pl, j]]))
                    j += 1
    for g in range(G):
        if not cand[g]:
            continue
        vmax = max(v for v, _ in cand[g])
        for v, p in cand[g]:
            if v == vmax:
                assigned[p] = g + 1
    return assigned


def _host_fallback(bb, gt):
    """Pure-numpy reference path (used only if a gt-group cap is hit)."""
    N = bb.shape[0]
    max_ov = np.zeros(N, np.float32)
    arg_ov = np.zeros(N, np.int64)
    gt_max = np.zeros(G, np.float32)
    area_g = (gt[:, 2] - gt[:, 0]) * (gt[:, 3] - gt[:, 1])
    area_b = (bb[:, 2] - bb[:, 0]) * (bb[:, 3] - bb[:, 1])
    step = 20000
    for s in range(0, N, step):
        e = min(s + step, N)
        lt = np.maximum(gt[:, None, :2], bb[None, s:e, :2])
        rb = np.minimum(gt[:, None, 2:], bb[None, s:e, 2:])
        wh = np.clip(rb - lt, 0, None).astype(np.float32)
        inter = wh[..., 0] * wh[..., 1]
        union = np.maximum(area_g[:, None] + area_b[None, s:e] - inter,
                           np.float32(1e-6))
        ov = inter / union
        max_ov[s:e] = ov.max(axis=0)
        arg_ov[s:e] = ov.argmax(axis=0)
        gt_max = np.maximum(gt_max, ov.max(axis=1))
        del lt, rb, wh, inter, union, ov
    assigned = np.where(max_ov >= 0.5, arg_ov + 1, 0).astype(np.int32)
    for s in range(0, N, step):
        e = min(s + step, N)
        lt = np.maximum(gt[:, None, :2], bb[None, s:e, :2])
        rb = np.minimum(gt[:, None, 2:], bb[None, s:e, 2:])
        wh = np.clip(rb - lt, 0, None).astype(np.float32)
        inter = wh[..., 0] * wh[..., 1]
        union = np.maximum(area_g[:, None] + area_b[None, s:e] - inter,
                           np.float32(1e-6))
        ov = inter / union
        eq = ov == gt_max[:, None]
        gidx = np.where(eq, np.arange(G)[:, None], -1).max(axis=0)
        sel = gidx >= 0
        assigned[s:e][sel] = gidx[sel] + 1
        del lt, rb, wh, inter, union, ov
    return assigned


def kernel(bboxes: np.ndarray, gt_bboxes: np.ndarray) -> np.ndarray:
    assert bboxes.shape == (N_FULL, 4) and gt_bboxes.shape == (G, 4)
    bb = np.ascontiguousarray(bboxes, dtype=np.float32)
    gt = np.ascontiguousarray(gt_bboxes, dtype=np.float32)
    prep = prepare_inputs(bb, gt)
    if prep is None:
        return _host_fallback(bb, gt)
    in_maps, meta = prep
    nc = _get_program()
    res = bass_utils.run_bass_kernel_spmd(nc, in_maps,
                                          core_ids=list(range(N_CORES)))
    return assemble(res, meta)


if __name__ == "__main__":
    rng = np.random.default_rng(0)
    bb_ = np.zeros((N_FULL, 4), np.float32)
    bb_[:, :2] = rng.uniform(0, 928, (N_FULL, 2))
    bb_[:, 2:] = bb_[:, :2] + rng.uniform(1, 97, (N_FULL, 2))
    gtb = np.zeros((G, 4), np.float32)
    gtb[:, :2] = rng.uniform(0, 928, (G, 2))
    gtb[:, 2:] = gtb[:, :2] + rng.uniform(1, 97, (G, 2))
    print(kernel(bb_, gtb)[:20])


# revision 10
# speedup vs baseline: 8.7234x; 1.2349x over previous
"""MaxIoUAssigner on 8 Trainium2 NeuronCores (Bass/Tile) — v3.

kernel(bboxes[200000,4] f32, gt_bboxes[256,4] f32) -> assigned[200000] int32

Semantics (mmdet MaxIoUAssigner, pos=neg=0.5, min_pos_iou=0,
gt_max_assign_all=True):
  overlaps = iou(gt, priors)  [256, 200000]
  per-prior max/argmax (first index wins ties); <0.5 -> 0; >=0.5 -> argmax+1
  low-quality: priors attaining a gt's row max get gt_i+1 (later gt wins)

This target's cost model is dominated by a flat ~41us per *instruction*
(engine-independent, nearly size-independent); DMA bytes overlap compute.
The kernel is therefore architected to minimize instruction count:

 - Priors are sorted by x on the host and split into 8 contiguous x-bands
   (one per core, 25000 each); each band is split by y into two halves.
   Each half only interacts with the <=64 gts whose boxes can touch it
   (data-checked; host falls back to a numpy path if a cap is exceeded).
 - On-device layout: 128 partitions = 2 groups x 64 gts; free dim = the
   half's full 12500 priors in ONE chunk.  Coordinates roll through a
   single [128,2,12500] buffer (x-pair, then y-pair, then area), so the
   whole IoU core is 8 fat instructions on [128,12500] operands.
 - Per-prior argmax+max: iou is encoded as key = floor(iou*2^15) +
   (63-p)/64 (exact in f32: 16+6 bits) and max-reduced across partitions
   with gpsimd partition_all_reduce per group; the host decodes
   (bucketed argmax, exact 0.5 threshold via the floor bias).
 - Per-gt max+argmax (for the low-quality step) via the DVE max/max_index
   top-8 instruction pair; candidates are combined on the host across
   halves/cores, which removes the iou stash, the whole second phase,
   and the gt-max AllReduce collective of the v1 design.
 - ~25 instructions per core vs ~250 in v1.

Host does only O(N) label decode + argsort; all 51.2M-element IoU work
runs on device.
"""

import sys

if "/opt/trn_rl_repo" not in sys.path:
    sys.path.insert(0, "/opt/trn_rl_repo")

import numpy as np

from concourse import bacc, bass_utils, mybir, tile

f32 = mybir.dt.float32
i32 = mybir.dt.int32
u32 = mybir.dt.uint32
Alu = mybir.AluOpType
ActF = mybir.ActivationFunctionType

N_FULL = 200000
G = 256
P = 128
HG = 64                      # gts per group (2 groups of 64 partitions)
N_CORES = 8
NB = N_FULL // N_CORES       # 25000 priors per core (x-band)
NH = NB // 2                 # 12500 per y-half (one chunk)
F = NH
KSCALE = 32768.0             # 2^15 iou quantization for the key encode
KTHR = 16384                 # floor(iou*2^15) >= 16384  <=>  iou >= 0.5
DUMMY = 1.0e8                # far-away dummy gt coordinate


def build_program(repeat=1, n_cores=N_CORES):
    import concourse.bass_isa as bass_isa

    nc = bacc.Bacc("TRN2", target_bir_lowering=False, debug=False,
                   num_devices=n_cores)
    # rows per half: x1, x2, y1, y2, area
    bbx = nc.dram_tensor("bbx", [2, 5, NH], f32, kind="ExternalInput").ap()
    gtc_d = nc.dram_tensor("gtc", [P, 8], f32, kind="ExternalInput").ap()
    okey = nc.dram_tensor("okey", [2, NH], f32, kind="ExternalOutput").ap()
    ogvi = nc.dram_tensor("ogvi", [P, 16], u32, kind="ExternalOutput").ap()

    with tile.TileContext(nc) as tc:
        with (
            tc.tile_pool(name="c", bufs=1) as cpool,
            tc.tile_pool(name="w", bufs=1) as wpool,
        ):
            gtc = cpool.tile([P, 8], f32, tag="gtc")
            gvi = cpool.tile([P, 16], u32, tag="gvi")
            # slots 1-7 of the value block stay at -1e30 so max_index only
            # resolves slot 0 (the true row max from tensor_reduce)
            nc.gpsimd.memset(gvi.bitcast(f32)[:, 0:8], -1.0e30)

            nc.sync.dma_start(gtc[:], gtc_d)
            gx1, gx2 = gtc[:, 0:1], gtc[:, 1:2]
            gy1, gy2 = gtc[:, 2:3], gtc[:, 3:4]
            gar, frac = gtc[:, 4:5], gtc[:, 5:6]

            for _rep in range(repeat):
                pair = wpool.tile([P, 2, F], f32, tag="pair")  # 100 KB
                io = wpool.tile([P, F], f32, tag="io")         # 50 KB
                tmp = wpool.tile([P, F], f32, tag="tmp")       # 50 KB

                def ld(r0, r1, dst, dw):
                    # broadcast rows [r0:r1) of each half to its 64 parts
                    for h in range(2):
                        nc.sync.dma_start(
                            dst[h * HG:(h + 1) * HG, 0:dw],
                            bbx[h, r0:r1].rearrange("r n -> () r n")
                            .broadcast_to([HG, dw, F]))

                ld(0, 2, pair, 2)                              # x1, x2
                nc.vector.tensor_scalar(tmp[:], pair[:, 0], gx1, None,
                                        op0=Alu.max)
                nc.vector.scalar_tensor_tensor(io[:], pair[:, 1], gx2,
                                               tmp[:], op0=Alu.min,
                                               op1=Alu.subtract)
                ld(2, 4, pair, 2)                              # y1, y2
                nc.vector.tensor_scalar(tmp[:], pair[:, 0], gy1, None,
                                        op0=Alu.max)
                nc.vector.scalar_tensor_tensor(tmp[:], pair[:, 1], gy2,
                                               tmp[:], op0=Alu.min,
                                               op1=Alu.subtract)
                nc.vector.scalar_tensor_tensor(io[:], io[:], 0.0, tmp[:],
                                               op0=Alu.max, op1=Alu.mult)
                ld(4, 5, pair, 1)                              # area
                nc.vector.scalar_tensor_tensor(tmp[:], pair[:, 0], gar,
                                               io[:], op0=Alu.add,
                                               op1=Alu.subtract)
                nc.vector.reciprocal(tmp[:], tmp[:])
                nc.vector.tensor_mul(io[:], io[:], tmp[:])
                # per-gt row max + its index for the low-quality step
                nc.vector.tensor_reduce(gvi.bitcast(f32)[:, 0:1], io[:],
                                        axis=mybir.AxisListType.X,
                                        op=Alu.max)
                nc.vector.max_index(gvi[:, 8:16],
                                    gvi.bitcast(f32)[:, 0:8], io[:])
                # per-prior key encode: floor(iou*2^15) + (63-p)/64
                ki = wpool.tile([P, F], i32, tag="pair")  # reuse pair slot
                nc.scalar.activation(ki[:], io[:], ActF.Copy, bias=-0.5,
                                     scale=KSCALE)
                nc.scalar.activation(tmp[:], ki[:], ActF.Relu, bias=frac,
                                     scale=1.0)
                # group max across partitions (AR ucode is base-0 only:
                # copy group B down to partition 0; io's slot is free now)
                tmp2 = wpool.tile([HG, F], f32, tag="io")
                nc.sync.dma_start(tmp2[:], tmp[HG:P])
                nc.gpsimd.partition_all_reduce(
                    tmp[0:HG], tmp[0:HG], channels=HG,
                    reduce_op=bass_isa.ReduceOp.max)
                nc.gpsimd.partition_all_reduce(
                    tmp2[:], tmp2[:], channels=HG,
                    reduce_op=bass_isa.ReduceOp.max)
                nc.sync.dma_start(okey[0:1, :], tmp[0:1, :])
                nc.sync.dma_start(okey[1:2, :], tmp2[0:1, :])
            nc.sync.dma_start(ogvi, gvi[:])
    nc.compile()
    return nc


_NC_CACHE = None


def _get_program():
    global _NC_CACHE
    if _NC_CACHE is None:
        _NC_CACHE = build_program()
    return _NC_CACHE


def prepare_inputs(bb, gt):
    """Sort priors into 8 x-bands x 2 y-halves; pick each half's gts.

    Returns (in_maps, meta) where meta[k] = (halves_idx, gmaps):
    halves_idx[h] = global prior indices of half h (device column order),
    gmaps[h] = ascending global gt indices assigned to that half's group.
    Returns None if a gt group exceeds HG (caller falls back).
    """
    xorder = np.argsort(bb[:, 0], kind="stable")
    in_maps, meta = [], []
    for k in range(N_CORES):
        band_idx = xorder[k * NB:(k + 1) * NB]
        yord = np.argsort(bb[band_idx, 1], kind="stable")
        halves = [band_idx[yord[:NH]], band_idx[yord[NH:]]]
        bbx = np.empty((2, 5, NH), np.float32)
        gtc = np.zeros((P, 8), np.float32)
        gmaps = []
        for h in range(2):
            B = bb[halves[h]]
            bbx[h, 0] = B[:, 0]
            bbx[h, 1] = B[:, 2]
            bbx[h, 2] = B[:, 1]
            bbx[h, 3] = B[:, 3]
            bbx[h, 4] = (B[:, 2] - B[:, 0]) * (B[:, 3] - B[:, 1])
            sel = np.nonzero(
                (gt[:, 0] <= B[:, 2].max()) & (gt[:, 2] >= B[:, 0].min())
                & (gt[:, 1] <= B[:, 3].max()) & (gt[:, 3] >= B[:, 1].min())
            )[0]
            if len(sel) > HG:
                return None
            base = h * HG
            n = len(sel)
            gtc[base:base + n, 0] = gt[sel, 0]
            gtc[base:base + n, 1] = gt[sel, 2]
            gtc[base:base + n, 2] = gt[sel, 1]
            gtc[base:base + n, 3] = gt[sel, 3]
            gtc[base:base + n, 4] = ((gt[sel, 2] - gt[sel, 0])
                                     * (gt[sel, 3] - gt[sel, 1]))
            gtc[base + n:base + HG, 0] = DUMMY
            gtc[base + n:base + HG, 1] = DUMMY + 1.0
            gtc[base + n:base + HG, 2] = DUMMY
            gtc[base + n:base + HG, 3] = DUMMY + 1.0
            gtc[base + n:base + HG, 4] = 1.0
            gtc[base:base + HG, 5] = (HG - 1 - np.arange(HG)) / HG
            gmaps.append(sel)
        in_maps.append({"bbx": bbx, "gtc": gtc})
        meta.append((halves, gmaps))
    return in_maps, meta


def assemble(res, meta):
    """Decode per-prior keys + per-gt candidates into final labels."""
    assigned = np.zeros(N_FULL, np.int32)
    cand = [[] for _ in range(G)]  # per gt: list of (val, prior) candidates
    for k in range(N_CORES):
        halves, gmaps = meta[k]
        r = res.results[k]
        okey = r["okey"]                       # [2, NH]
        ogvi = r["ogvi"]                       # [P, 16] u32
        for h in range(2):
            v = okey[h].astype(np.float64)
            w = np.rint(v * HG).astype(np.int64)
            kib = w >> 6
            plocal = (HG - 1) - (w & (HG - 1))
            gsel = gmaps[h]
            gl = np.full(HG, -1, np.int64)
            gl[:len(gsel)] = gsel
            gwin = gl[np.clip(plocal, 0, HG - 1)]
            lab = np.where((kib >= KTHR) & (gwin >= 0), gwin + 1, 0)
            assigned[halves[h]] = lab
        # gt-side candidates
        val = ogvi[:, 0:8].view(np.float32)
        idx = ogvi[:, 8:16]
        for h in range(2):
            gsel = gmaps[h]
            if not len(gsel):
                continue
            base = h * HG
            pri = halves[h]
            for pl, g in enumerate(gsel):
                v0 = val[base + pl, 0]
                cand[g].append((v0, pri[idx[base + pl, 0]]))
                # exact ties within this half's top-8
                j = 1
                while j < 8 and val[base + pl, j] == v0:
                    cand[g].append((v0, pri[idx[base + pl, j]]))
                    j += 1
    for g in range(G):
        if not cand[g]:
            continue
        vmax = max(v for v, _ in cand[g])
        for v, p in cand[g]:
            if v == vmax:
                assigned[p] = g + 1
    return assigned


def _host_fallback(bb, gt):
    """Pure-numpy reference path (used only if a gt-group cap is hit)."""
    N = bb.shape[0]
    max_ov = np.zeros(N, np.float32)
    arg_ov = np.zeros(N, np.int64)
    gt_max = np.zeros(G, np.float32)
    area_g = (gt[:, 2] - gt[:, 0]) * (gt[:, 3] - gt[:, 1])
    area_b = (bb[:, 2] - bb[:, 0]) * (bb[:, 3] - bb[:, 1])
    step = 20000
    for s in range(0, N, step):
        e = min(s + step, N)
        lt = np.maximum(gt[:, None, :2], bb[None, s:e, :2])
        rb = np.minimum(gt[:, None, 2:], bb[None, s:e, 2:])
        wh = np.clip(rb - lt, 0, None).astype(np.float32)
        inter = wh[..., 0] * wh[..., 1]
        union = np.maximum(area_g[:, None] + area_b[None, s:e] - inter,
                           np.float32(1e-6))
        ov = inter / union
        max_ov[s:e] = ov.max(axis=0)
        arg_ov[s:e] = ov.argmax(axis=0)
        gt_max = np.maximum(gt_max, ov.max(axis=1))
        del lt, rb, wh, inter, union, ov
    assigned = np.where(max_ov >= 0.5, arg_ov + 1, 0).astype(np.int32)
    for s in range(0, N, step):
        e = min(s + step, N)
        lt = np.maximum(gt[:, None, :2], bb[None, s:e, :2])
        rb = np.minimum(gt[:, None, 2:], bb[None, s:e, 2:])
        wh = np.clip(rb - lt, 0, None).astype(np.float32)
        inter = wh[..., 0] * wh[..., 1]
        union = np.maximum(area_g[:, None] + area_b[None, s:e] - inter,
                           np.float32(1e-6))
        ov = inter / union
        eq = ov == gt_max[:, None]
        gidx = np.where(eq, np.arange(G)[:, None], -1).max(axis=0)
        sel = gidx >= 0
        assigned[s:e][sel] = gidx[sel] + 1
        del lt, rb, wh, inter, union, ov
    return assigned


def kernel(bboxes: np.ndarray, gt_bboxes: np.ndarray) -> np.ndarray:
    assert bboxes.shape == (N_FULL, 4) and gt_bboxes.shape == (G, 4)
    bb = np.ascontiguousarray(bboxes, dtype=np.float32)
    gt = np.ascontiguousarray(gt_bboxes, dtype=np.float32)
    prep = prepare_inputs(bb, gt)
    if prep is None:
        return _host_fallback(bb, gt)
    in_maps, meta = prep
    nc = _get_program()
    res = bass_utils.run_bass_kernel_spmd(nc, in_maps,
                                          core_ids=list(range(N_CORES)))
    return assemble(res, meta)


if __name__ == "__main__":
    rng = np.random.default_rng(0)
    bb_ = np.zeros((N_FULL, 4), np.float32)
    bb_[:, :2] = rng.uniform(0, 928, (N_FULL, 2))
    bb_[:, 2:] = bb_[:, :2] + rng.uniform(1, 97, (N_FULL, 2))
    gtb = np.zeros((G, 4), np.float32)
    gtb[:, :2] = rng.uniform(0, 928, (G, 2))
    gtb[:, 2:] = gtb[:, :2] + rng.uniform(1, 97, (G, 2))
    print(kernel(bb_, gtb)[:20])
